# revision 50
# baseline (speedup 1.0000x reference)
"""Trainium2 Bass kernel for a single causal attention head (v3).

Problem: x [8, 2048, 1024] f32, Wq/Wk/Wv [1024, 64] f32.
out[b] = softmax(causal(x[b] Wq (x[b] Wk)^T) / 8) @ (x[b] Wv)   -> [8, 2048, 64] f32

Sharding: data-parallel over batch. Each of the 8 NeuronCores runs the same
single-core program on its own batch element (no collectives).

Per-body dataflow (matmuls in bf16, fp32 PSUM accumulation):
  1. x loads in 16 single-s-tile SWDGE cast-DMAs (f32->bf16).  Weights load
     f32 via sync HWDGE + DVE cast so the Pool SWDGE queue stays on x.
  2. x^T via PE matmul-by-identity, per s-tile, psum copies alternate DVE/Act.
  3. Packed [Wq|Wk] stationary: one matmul per (q-chunk, d-chunk) produces
     Q^T and K^T together; V^T separate; V natural (+ones col) via PE.
  4. scoresT[k, q] exact-causal from column kt*128, exp on Act (1/8 folded
     in), diagonal-block trimask on DVE.
  5. attnT @ [V | ones] accumulated per 512-wide output quarter (walrus
     needs uniform element counts per psum accumulation group); all four
     quarters drain early inside their half's kt loop: O^T -> xbar
     transpose -> reciprocal-normalize -> DMA out.
  6. Cross-body skew: each attention half's kt loop interleaves "filler"
     units (transposes/projections of the NEXT body's frontend) into the
     PE stream, absorbing the PE idle slots of the Act-bound exp pipeline.

Timing loop: bodies are emitted back-to-back inside one For_i iteration
(20 per iteration), software-pipelined two-deep: body i's attention is
emitted with body i+1's frontend woven in, and per-body staging tiles are
2-deep pool slots so bodies alternate buffers.  The For_i all-engine
barrier only hits every `unroll` bodies.  Constants (identity, trimask,
act table) are set up once per NEFF execution.

Measured (time_hwloop marginal, 8 cores): baseline 92.5us (re-measured
92-99us) -> 56-80us across runs, ~67-75us typical; device timing variance
is +-10-20us for identical builds, so finer HW discrimination was not
possible.  TimelineSim: single-body 64us; consecutive-body marginals
44.8/51.6/51.0us (the pipeline converges to ~51us/body as startup slack
drains; vnat chunk placement inside the h0 kt loop, not at its head, is
load-bearing for that convergence).  PSUM: frontend tag bufs=4, scores
bufs=2 (av_lag=3 covers exp latency), opsum bufs=2 = 8 banks.  The For_i
loop body is fully cross-boundary skewed (the last body preps the next
iteration's first; per-iteration tile allocations stay slot-consistent for
even unroll, and data is body-invariant so odd-unroll fallbacks remain
value-correct).  The sim does not model Ldweights issue cost (~274 pairs
per body), which accounts for most of the HW gap.  The converged-state
limiter (one periodic 7.2us PE gap per body) is the x-load intake: Pool's
SWDGE descgen blocks ~37us inside one DMACopy per body waiting on load
completion semaphores (not ring capacity - ring size and x_bf depth are
both neutral); unblocking it would need the x intake moved off the
serialized SWDGE path, e.g. HWDGE f32 loads + engine casts.
"""

import math
import sys

import numpy as np

if "/opt/trn_rl_repo" not in sys.path:
    sys.path.insert(0, "/opt/trn_rl_repo")

import concourse.bacc as bacc
import concourse.tile as tile
from concourse import mybir
from concourse.masks import make_identity

BATCH = 8
SEQ = 2048
D_EMBED = 1024
HEAD = 64
N_CORES = 8

F32 = mybir.dt.float32
BF16 = mybir.dt.bfloat16


def build_attention_nc(S=SEQ, D=D_EMBED, repeat=1, phase="full",
                       sp_w=512, fps_bufs=4, sp_bufs=2, nbody=1, unroll=20,
                       av_lag=3, atn_bufs=5, pool_copy=False, h0_per_kt=2,
                       dma_ring=16384, n_swq=1, xbf_bufs=2, qk_bufs=2,
                       sp_share=False, hyb_load=False):
    """Build the single-core Bass program for one batch element."""
    H = HEAD
    ST = S // 128          # s-tiles (16)
    DC = D // 128          # d-chunks (8)
    QW = 512               # q-chunk width
    HW_ = S // 2           # half width (1024)
    inv_sqrt_h = 1.0 / math.sqrt(H)

    nc = bacc.Bacc("TRN2", target_bir_lowering=False, debug=False,
               dynamic_dma_scratch_size=dma_ring,
               num_swdge_queues=n_swq)

    x_dram = nc.dram_tensor("x", [S, D], F32, kind="ExternalInput").ap()
    wq_dram = nc.dram_tensor("Wq", [D, H], F32, kind="ExternalInput").ap()
    wk_dram = nc.dram_tensor("Wk", [D, H], F32, kind="ExternalInput").ap()
    wv_dram = nc.dram_tensor("Wv", [D, H], F32, kind="ExternalInput").ap()
    out_dram = nc.dram_tensor("out", [S, H], F32, kind="ExternalOutput").ap()
    out_r = out_dram.rearrange("(t p) h -> p t h", p=128)

    if repeat > 1 and repeat % unroll:
        unroll = next(u for u in (20, 10, 8, 5, 4, 2, 1)
                      if repeat % u == 0)

    with tile.TileContext(nc) as tc:
        with (
            tc.tile_pool(name="sb", bufs=1) as sb,
            tc.tile_pool(name="fps", bufs=fps_bufs, space="PSUM") as fps,
            tc.tile_pool(name="aps", bufs=1, space="PSUM") as aps,
            tc.tile_pool(name="atn", bufs=atn_bufs) as atn,
        ):
            # ---------------- persistent SBUF ----------------
            xt2 = sb.tile([128, ST * DC, 128], BF16)
            ident = sb.tile([128, 128], BF16)
            trimask = sb.tile([128, 128], BF16)
            wqk = sb.tile([128, DC, 128], BF16)   # [Wq | Wk] packed
            wv_sb = sb.tile([128, DC, H], BF16)
            wq_f = sb.tile([128, DC, H], F32)
            wk_f = sb.tile([128, DC, H], F32)
            wv_f = sb.tile([128, DC, H], F32)
            dum = sb.tile([128, 1], BF16)

            x_src = x_dram.rearrange("(a p) d -> p a d", p=128)

            # ------------- one-time constants -------------
            make_identity(nc, ident)
            # trimask[k_local, q_local] = 1.0 if q_local >= k_local else 0.0
            nc.gpsimd.memset(trimask, 1.0)
            nc.gpsimd.affine_select(
                out=trimask,
                in_=trimask,
                compare_op=mybir.AluOpType.is_ge,
                fill=0.0,
                base=0,
                pattern=[[1, 128]],
                channel_multiplier=-1,
            )
            # preload the Exp activation table while everything idles
            nc.scalar.activation(
                out=dum, in_=ident[:, 0:1],
                func=mybir.ActivationFunctionType.Exp,
            )

            def alloc_body_tiles():
                T = {}
                T["x_bf"] = sb.tile([128, ST, D], BF16, name="x_bf",
                                    tag="xbf", bufs=xbf_bufs)
                T["qk_a"] = sb.tile([128, S], BF16, name="qk_a",
                                    tag="qka", bufs=qk_bufs)
                T["qk_b"] = sb.tile([128, S], BF16, name="qk_b",
                                    tag="qkb", bufs=qk_bufs)
                T["vt"] = sb.tile([64, S], BF16, name="vt", tag="vt", bufs=qk_bufs)
                T["vnat"] = sb.tile([128, ST, H + 1], BF16, name="vnat",
                                    tag="vnat", bufs=qk_bufs)
                T["osb"] = sb.tile([80, S], BF16, name="osb",
                                   tag="osb", bufs=2)
                T["onat"] = sb.tile([128, ST, 80], BF16, name="onat",
                                    tag="onat", bufs=2)
                T["o_out"] = sb.tile([128, ST, H], F32, name="o_out",
                                     tag="oo", bufs=2)
                T["rcp"] = sb.tile([128, ST], F32, name="rcp",
                                   tag="rcp", bufs=2)
                nc.vector.memset(T["vnat"][:, :, H:H + 1], 1.0)
                nc.vector.memset(T["osb"][64:80, :], 0.0)
                # intake: most chunks via SWDGE cast-DMA; every 4th via
                # sync HWDGE f32 + DVE cast to relieve the SWDGE path
                for c in range(ST):
                    if c % 4 != 3 or not hyb_load:
                        nc.gpsimd.dma_start(out=T["x_bf"][:, c, :],
                                            in_=x_src[:, c, :])
                    else:
                        stg = sb.tile([128, D], F32, name="xstg",
                                      tag="xstg", bufs=2)
                        nc.sync.dma_start(out=stg, in_=x_src[:, c, :])
                        nc.vector.tensor_copy(T["x_bf"][:, c, :], stg)
                for wf, wd in ((wq_f, wq_dram), (wk_f, wk_dram),
                               (wv_f, wv_dram)):
                    nc.sync.dma_start(
                        out=wf, in_=wd.rearrange("(j p) h -> p j h", p=128)
                    )
                nc.vector.tensor_copy(wqk[:, :, 0:H], wq_f)
                nc.vector.tensor_copy(wqk[:, :, H:128], wk_f)
                nc.vector.tensor_copy(wv_sb, wv_f)
                return T

            # ---------------- frontend helpers ----------------
            def xt_rhs(j, qc):
                # [128, 4, 128]: x^T d-chunk j for q-chunk qc
                return xt2[:, qc * 4 * DC + j:(qc + 1) * 4 * DC:DC, :]

            def emit_xpose(T, si):
                for g in range(2):
                    xp = fps.tile([128, 512], F32, name="xp", tag="f")
                    for k in range(4):
                        j = g * 4 + k
                        nc.tensor.matmul(
                            xp[:, k * 128:(k + 1) * 128],
                            lhsT=T["x_bf"][:, si, j * 128:(j + 1) * 128],
                            rhs=ident,
                            start=True, stop=True,
                        )
                    dst = xt2[:, si * DC + g * 4:si * DC + g * 4 + 4, :]
                    if g % 2 == 0:
                        nc.vector.tensor_copy(dst, xp)
                    else:
                        nc.scalar.copy(dst, xp)

            def emit_proj(T, qc):
                qsl = slice(qc * QW, (qc + 1) * QW)
                pp = fps.tile([128, QW], F32, name="pp", tag="f")
                for j in range(DC):
                    nc.tensor.matmul(
                        pp, lhsT=wqk[:, j, :], rhs=xt_rhs(j, qc),
                        start=(j == 0), stop=(j == DC - 1),
                    )
                nc.vector.tensor_copy(T["qk_a"][:, qsl], pp)
                nc.sync.dma_start(out=T["qk_b"][0:64, qsl],
                                  in_=T["qk_a"][64:128, qsl])
                nc.sync.dma_start(out=T["qk_b"][64:128, qsl],
                                  in_=T["qk_a"][0:64, qsl])
                pv = fps.tile([128, QW], F32, name="pv", tag="f")
                for j in range(DC):
                    nc.tensor.matmul(
                        pv[0:64, :], lhsT=wv_sb[:, j, :], rhs=xt_rhs(j, qc),
                        start=(j == 0), stop=(j == DC - 1),
                    )
                nc.vector.tensor_copy(T["vt"][:, qsl], pv[0:64, :])

            def emit_vnat(T, qc):
                vp = fps.tile([128, 4, H], F32, name="vp", tag="f")
                for t in range(4):
                    st = qc * 4 + t
                    nc.tensor.matmul(
                        vp[:, t, :],
                        lhsT=T["vt"][:, st * 128:(st + 1) * 128],
                        rhs=ident[0:64, 0:64],
                        start=True, stop=True,
                    )
                nc.scalar.copy(T["vnat"][:, qc * 4:(qc + 1) * 4, 0:H], vp)

            # ---------------- attention helpers ----------------
            def scores_mm(T, kt, dst, c0, c1):
                col = slice(kt * 128, (kt + 1) * 128)
                if kt % 2 == 0:
                    nc.tensor.matmul(
                        dst, lhsT=T["qk_b"][0:64, col],
                        rhs=T["qk_a"][0:64, c0:c1],
                        start=True, stop=True,
                    )
                else:
                    nc.tensor.matmul(
                        dst, lhsT=T["qk_a"][64:128, col],
                        rhs=T["qk_b"][64:128, c0:c1],
                        start=True, stop=True,
                    )

            def finalize_q(T, q, opsum):
                # drain quarter q (columns [q*512, (q+1)*512)) of O^T
                q_lo = q * QW
                nc.vector.tensor_copy(T["osb"][0:H + 1, q_lo:q_lo + QW], opsum)
                nc.sync.dma_start(
                    out=T["onat"][:, q * 4:(q + 1) * 4, :],
                    in_=T["osb"][0:80, q_lo:q_lo + QW],
                    transpose=True,
                )
                for t in range(q * 4, (q + 1) * 4):
                    nc.vector.reciprocal(T["rcp"][:, t:t + 1],
                                         T["onat"][:, t, H:H + 1])
                    nc.vector.tensor_scalar_mul(
                        T["o_out"][:, t, :], T["onat"][:, t, 0:H],
                        T["rcp"][:, t:t + 1]
                    )
                nc.sync.dma_start(
                    out=out_r[:, q * 4:(q + 1) * 4, :],
                    in_=T["o_out"][:, q * 4:(q + 1) * 4, :],
                )

            def attn_half(T, h, fillers=(), per_kt=1, interleave=None,
                          drain_early=False):
                h_lo, h_hi = h * HW_, (h + 1) * HW_
                n_kt = h_hi // 128
                fillers = list(fillers)
                ops = {}
                for q in (2 * h, 2 * h + 1):
                    ops[q] = aps.tile([H + 1, QW], F32, name="opsum",
                                      tag="o", bufs=2)

                def emit_attnV(kt, at):
                    for q in (2 * h, 2 * h + 1):
                        if kt // 4 > q:
                            continue
                        nc.tensor.matmul(
                            ops[q],
                            lhsT=T["vnat"][:, kt, :],
                            rhs=at[:, q * QW:(q + 1) * QW],
                            start=(kt == 0),
                            stop=(kt == 4 * q + 3),
                            skip_group_check=True,
                        )
                    if drain_early and kt == 8 * h + 3:
                        finalize_q(T, 2 * h, ops[2 * h])

                pending = []
                for kt in range(n_kt):
                    lo = max(h_lo, kt * 128)
                    at = atn.tile([128, S], BF16, name="attn", tag="at")
                    c = lo
                    while c < h_hi:
                        ce = min(c + sp_w, h_hi)
                        if sp_share:
                            sp = fps.tile([128, sp_w], F32, name="sp",
                                          tag="f")
                        else:
                            sp = aps.tile([128, sp_w], F32, name="sp",
                                          tag="s", bufs=sp_bufs)
                        scores_mm(T, kt, sp[:, 0:ce - c], c, ce)
                        nc.scalar.activation(
                            out=at[:, c:ce], in_=sp[:, 0:ce - c],
                            func=mybir.ActivationFunctionType.Exp,
                            scale=inv_sqrt_h,
                        )
                        c = ce
                    if kt * 128 >= h_lo:  # diagonal block in this half
                        nc.vector.tensor_mul(
                            at[:, kt * 128:(kt + 1) * 128],
                            at[:, kt * 128:(kt + 1) * 128],
                            trimask,
                        )
                        if kt % 4:  # zero [dq*512, kt*128) below-diag cols
                            nc.vector.memset(
                                at[:, (kt // 4) * QW:kt * 128], 0.0
                            )
                    pending.append((kt, at))
                    if len(pending) > av_lag:
                        emit_attnV(*pending.pop(0))
                    for _ in range(per_kt):
                        if fillers:
                            fillers.pop(0)()
                    if interleave is not None and kt == 2:
                        interleave()
                for p in pending:
                    emit_attnV(*p)
                for u in fillers:
                    u()
                if drain_early:
                    finalize_q(T, 2 * h + 1, ops[2 * h + 1])
                return ops

            # ---------------- body sequencing (skewed) ----------------
            def front1_units(T):
                return ([(lambda si=si: emit_xpose(T, si))
                         for si in range(8)]
                        + [lambda: emit_proj(T, 0), lambda: emit_proj(T, 1),
                           lambda: emit_vnat(T, 0)])

            def front2_units(T):
                return ([(lambda si=si: emit_xpose(T, si))
                         for si in range(8, 16)]
                        + [lambda: emit_proj(T, 2), lambda: emit_proj(T, 3),
                           lambda: emit_vnat(T, 2)])

            def emit_bodies(n, looping=False):
                T = alloc_body_tiles()
                if phase == "load":
                    for t in range(ST):
                        nc.vector.tensor_copy(
                            T["o_out"][:, t, :],
                            T["x_bf"].rearrange("p a b -> p (a b)")
                                [:, t * H:(t + 1) * H],
                        )
                    nc.sync.dma_start(out=out_r, in_=T["o_out"])
                    return
                for u in front1_units(T):
                    u()
                for b in range(n):
                    attn_half(
                        T, 0, fillers=front2_units(T), per_kt=h0_per_kt,
                        interleave=lambda TT=T: emit_vnat(TT, 1),
                        drain_early=True,
                    )
                    if b < n - 1 or looping:
                        Tn = alloc_body_tiles()
                        attn_half(T, 1,
                                  fillers=[lambda TT=T: emit_vnat(TT, 3)]
                                          + front1_units(Tn),
                                  drain_early=True)
                        T = Tn
                    else:
                        attn_half(T, 1,
                                  fillers=[lambda TT=T: emit_vnat(TT, 3)],
                                  drain_early=True)

            def emit_loop_bodies(n):
                # cross-boundary skew: every body fully skewed; the last
                # body preps the next iteration's first body before the
                # For_i back-edge.  Requires per-iteration pool-slot counts
                # divisible by their bufs (holds: 10 bodies/iteration).
                for b in range(n):
                    T = LC["T"]
                    attn_half(
                        T, 0, fillers=front2_units(T), per_kt=h0_per_kt,
                        interleave=lambda TT=T: emit_vnat(TT, 1),
                        drain_early=True,
                    )
                    Tn = alloc_body_tiles()
                    attn_half(T, 1,
                              fillers=[lambda TT=T: emit_vnat(TT, 3)]
                                      + front1_units(Tn),
                              drain_early=True)
                    LC["T"] = Tn

            if repeat > 1:
                LC = {"T": alloc_body_tiles()}
                for u in front1_units(LC["T"]):
                    u()
                with tc.For_i(0, repeat // unroll, 1):
                    emit_loop_bodies(unroll)
            else:
                emit_bodies(nbody)
    nc.compile()
    return nc


_NC_CACHE = {}


def _get_nc(S=SEQ, D=D_EMBED):
    key = (S, D)
    if key not in _NC_CACHE:
        _NC_CACHE[key] = build_attention_nc(S, D)
    return _NC_CACHE[key]


def kernel(x, Wq, Wk, Wv):
    """Full-input entry point: x [8, 2048, 1024] f32 -> [8, 2048, 64] f32."""
    from concourse.bass_utils import run_bass_kernel_spmd

    x = np.asarray(x, dtype=np.float32)
    Wq = np.ascontiguousarray(np.asarray(Wq, dtype=np.float32))
    Wk = np.ascontiguousarray(np.asarray(Wk, dtype=np.float32))
    Wv = np.ascontiguousarray(np.asarray(Wv, dtype=np.float32))
    assert x.shape == (BATCH, SEQ, D_EMBED), x.shape

    nc = _get_nc()
    in_maps = [
        {"x": np.ascontiguousarray(x[b]), "Wq": Wq, "Wk": Wk, "Wv": Wv}
        for b in range(BATCH)
    ]
    res = run_bass_kernel_spmd(nc, in_maps, core_ids=list(range(N_CORES)))
    return np.stack([res.results[b]["out"] for b in range(BATCH)], axis=0)


# revision 51
# speedup vs baseline: 1.0509x; 1.0509x over previous
"""Trainium2 Bass kernel for a single causal attention head (v3).

Problem: x [8, 2048, 1024] f32, Wq/Wk/Wv [1024, 64] f32.
out[b] = softmax(causal(x[b] Wq (x[b] Wk)^T) / 8) @ (x[b] Wv)   -> [8, 2048, 64] f32

Sharding: data-parallel over batch. Each of the 8 NeuronCores runs the same
single-core program on its own batch element (no collectives).

Per-body dataflow (matmuls in bf16, fp32 PSUM accumulation):
  1. x loads in 16 single-s-tile SWDGE cast-DMAs (f32->bf16).  Weights load
     f32 via sync HWDGE + DVE cast so the Pool SWDGE queue stays on x.
  2. x^T via PE matmul-by-identity, per s-tile, psum copies alternate DVE/Act.
  3. Packed [Wq|Wk] stationary: one matmul per (q-chunk, d-chunk) produces
     Q^T and K^T together; V^T separate; V natural (+ones col) via PE.
  4. scoresT[k, q] exact-causal from column kt*128, exp on Act (1/8 folded
     in), diagonal-block trimask on DVE.
  5. attnT @ [V | ones] accumulated per 512-wide output quarter (walrus
     needs uniform element counts per psum accumulation group); all four
     quarters drain early inside their half's kt loop: O^T -> xbar
     transpose -> reciprocal-normalize -> DMA out.
  6. Cross-body skew: each attention half's kt loop interleaves "filler"
     units (transposes/projections of the NEXT body's frontend) into the
     PE stream, absorbing the PE idle slots of the Act-bound exp pipeline.

Timing loop: bodies are emitted back-to-back inside one For_i iteration
(20 per iteration), software-pipelined two-deep: body i's attention is
emitted with body i+1's frontend woven in, and per-body staging tiles are
2-deep pool slots so bodies alternate buffers.  The For_i all-engine
barrier only hits every `unroll` bodies.  Constants (identity, trimask,
act table) are set up once per NEFF execution.

Measured (time_hwloop marginal, 8 cores): baseline 92.5us (re-measured
92-99us) -> 56-80us across runs, ~67-75us typical; device timing variance
is +-10-20us for identical builds, so finer HW discrimination was not
possible.  TimelineSim: single-body 64us; consecutive-body marginals
44.8/51.6/51.0us (the pipeline converges to ~51us/body as startup slack
drains; vnat chunk placement inside the h0 kt loop, not at its head, is
load-bearing for that convergence).  PSUM: frontend tag bufs=4, scores
bufs=2 (av_lag=3 covers exp latency), opsum bufs=2 = 8 banks.  The For_i
loop body is fully cross-boundary skewed (the last body preps the next
iteration's first; per-iteration tile allocations stay slot-consistent for
even unroll, and data is body-invariant so odd-unroll fallbacks remain
value-correct).  The sim does not model Ldweights issue cost (~274 pairs
per body), which accounts for most of the HW gap.  The converged-state
limiter (one periodic 7.2us PE gap per body) is the x-load intake: Pool's
SWDGE descgen blocks ~37us inside one DMACopy per body waiting on load
completion semaphores (not ring capacity - ring size and x_bf depth are
both neutral); unblocking it would need the x intake moved off the
serialized SWDGE path, e.g. HWDGE f32 loads + engine casts.
"""

import math
import sys

import numpy as np

if "/opt/trn_rl_repo" not in sys.path:
    sys.path.insert(0, "/opt/trn_rl_repo")

import concourse.bacc as bacc
import concourse.tile as tile
from concourse import mybir
from concourse.masks import make_identity

BATCH = 8
SEQ = 2048
D_EMBED = 1024
HEAD = 64
N_CORES = 8

F32 = mybir.dt.float32
BF16 = mybir.dt.bfloat16


def build_attention_nc(S=SEQ, D=D_EMBED, repeat=1, phase="full",
                       sp_w=512, fps_bufs=4, sp_bufs=2, nbody=1, unroll=20,
                       av_lag=3, atn_bufs=5, pool_copy=False, h0_per_kt=2,
                       dma_ring=16384, n_swq=1, xbf_bufs=2, qk_bufs=2,
                       sp_share=False, hyb_load=False, lc=1):
    """Build the single-core Bass program for one batch element."""
    H = HEAD
    ST = S // 128          # s-tiles (16)
    DC = D // 128          # d-chunks (8)
    QW = 512               # q-chunk width
    HW_ = S // 2           # half width (1024)
    inv_sqrt_h = 1.0 / math.sqrt(H)

    nc = bacc.Bacc("TRN2", target_bir_lowering=False, debug=False,
               dynamic_dma_scratch_size=dma_ring,
               num_swdge_queues=n_swq)

    x_dram = nc.dram_tensor("x", [S, D], F32, kind="ExternalInput").ap()
    wq_dram = nc.dram_tensor("Wq", [D, H], F32, kind="ExternalInput").ap()
    wk_dram = nc.dram_tensor("Wk", [D, H], F32, kind="ExternalInput").ap()
    wv_dram = nc.dram_tensor("Wv", [D, H], F32, kind="ExternalInput").ap()
    out_dram = nc.dram_tensor("out", [S, H], F32, kind="ExternalOutput").ap()
    out_r = out_dram.rearrange("(t p) h -> p t h", p=128)

    if repeat > 1 and repeat % unroll:
        unroll = next(u for u in (20, 10, 8, 5, 4, 2, 1)
                      if repeat % u == 0)

    with tile.TileContext(nc) as tc:
        with (
            tc.tile_pool(name="sb", bufs=1) as sb,
            tc.tile_pool(name="fps", bufs=fps_bufs, space="PSUM") as fps,
            tc.tile_pool(name="aps", bufs=1, space="PSUM") as aps,
            tc.tile_pool(name="atn", bufs=atn_bufs) as atn,
        ):
            # ---------------- persistent SBUF ----------------
            xt2 = sb.tile([128, ST * DC, 128], BF16)
            ident = sb.tile([128, 128], BF16)
            trimask = sb.tile([128, 128], BF16)
            wqk = sb.tile([128, DC, 128], BF16)   # [Wq | Wk] packed
            wv_sb = sb.tile([128, DC, H], BF16)
            wq_f = sb.tile([128, DC, H], F32)
            wk_f = sb.tile([128, DC, H], F32)
            wv_f = sb.tile([128, DC, H], F32)
            dum = sb.tile([128, 1], BF16)

            x_src = x_dram.rearrange("(a p) d -> p a d", p=128)

            # ------------- one-time constants -------------
            make_identity(nc, ident)
            # trimask[k_local, q_local] = 1.0 if q_local >= k_local else 0.0
            nc.gpsimd.memset(trimask, 1.0)
            nc.gpsimd.affine_select(
                out=trimask,
                in_=trimask,
                compare_op=mybir.AluOpType.is_ge,
                fill=0.0,
                base=0,
                pattern=[[1, 128]],
                channel_multiplier=-1,
            )
            # preload the Exp activation table while everything idles
            nc.scalar.activation(
                out=dum, in_=ident[:, 0:1],
                func=mybir.ActivationFunctionType.Exp,
            )

            def alloc_body_tiles():
                T = {}
                T["x_bf"] = sb.tile([128, ST, D], BF16, name="x_bf",
                                    tag="xbf", bufs=xbf_bufs)
                T["qk_a"] = sb.tile([128, S], BF16, name="qk_a",
                                    tag="qka", bufs=qk_bufs)
                T["qk_b"] = sb.tile([128, S], BF16, name="qk_b",
                                    tag="qkb", bufs=qk_bufs)
                T["vt"] = sb.tile([64, S], BF16, name="vt", tag="vt", bufs=qk_bufs)
                T["vnat"] = sb.tile([128, ST, H + 1], BF16, name="vnat",
                                    tag="vnat", bufs=qk_bufs)
                T["osb"] = sb.tile([80, S], BF16, name="osb",
                                   tag="osb", bufs=2)
                T["onat"] = sb.tile([128, ST, 80], BF16, name="onat",
                                    tag="onat", bufs=2)
                T["o_out"] = sb.tile([128, ST, H], F32, name="o_out",
                                     tag="oo", bufs=2)
                T["rcp"] = sb.tile([128, ST], F32, name="rcp",
                                   tag="rcp", bufs=2)
                nc.vector.memset(T["vnat"][:, :, H:H + 1], 1.0)
                nc.vector.memset(T["osb"][64:80, :], 0.0)
                # intake: most chunks via SWDGE cast-DMA; every 4th via
                # sync HWDGE f32 + DVE cast to relieve the SWDGE path
                for c in range(ST):
                    if c % 4 != 3 or not hyb_load:
                        nc.gpsimd.dma_start(out=T["x_bf"][:, c, :],
                                            in_=x_src[:, c, :])
                    else:
                        stg = sb.tile([128, D], F32, name="xstg",
                                      tag="xstg", bufs=2)
                        nc.sync.dma_start(out=stg, in_=x_src[:, c, :])
                        nc.vector.tensor_copy(T["x_bf"][:, c, :], stg)
                for wf, wd in ((wq_f, wq_dram), (wk_f, wk_dram),
                               (wv_f, wv_dram)):
                    nc.sync.dma_start(
                        out=wf, in_=wd.rearrange("(j p) h -> p j h", p=128)
                    )
                nc.vector.tensor_copy(wqk[:, :, 0:H], wq_f)
                nc.vector.tensor_copy(wqk[:, :, H:128], wk_f)
                nc.vector.tensor_copy(wv_sb, wv_f)
                return T

            # ---------------- frontend helpers ----------------
            def xt_rhs(j, qc):
                # [128, 4, 128]: x^T d-chunk j for q-chunk qc
                return xt2[:, qc * 4 * DC + j:(qc + 1) * 4 * DC:DC, :]

            def emit_xpose(T, si):
                for g in range(2):
                    xp = fps.tile([128, 512], F32, name="xp", tag="f")
                    for k in range(4):
                        j = g * 4 + k
                        nc.tensor.matmul(
                            xp[:, k * 128:(k + 1) * 128],
                            lhsT=T["x_bf"][:, si, j * 128:(j + 1) * 128],
                            rhs=ident,
                            start=True, stop=True,
                        )
                    dst = xt2[:, si * DC + g * 4:si * DC + g * 4 + 4, :]
                    if g % 2 == 0:
                        nc.vector.tensor_copy(dst, xp)
                    else:
                        nc.scalar.copy(dst, xp)

            def emit_proj(T, qc):
                qsl = slice(qc * QW, (qc + 1) * QW)
                pp = fps.tile([128, QW], F32, name="pp", tag="f")
                for j in range(DC):
                    nc.tensor.matmul(
                        pp, lhsT=wqk[:, j, :], rhs=xt_rhs(j, qc),
                        start=(j == 0), stop=(j == DC - 1),
                    )
                nc.vector.tensor_copy(T["qk_a"][:, qsl], pp)
                nc.sync.dma_start(out=T["qk_b"][0:64, qsl],
                                  in_=T["qk_a"][64:128, qsl])
                nc.sync.dma_start(out=T["qk_b"][64:128, qsl],
                                  in_=T["qk_a"][0:64, qsl])
                pv = fps.tile([128, QW], F32, name="pv", tag="f")
                for j in range(DC):
                    nc.tensor.matmul(
                        pv[0:64, :], lhsT=wv_sb[:, j, :], rhs=xt_rhs(j, qc),
                        start=(j == 0), stop=(j == DC - 1),
                    )
                nc.vector.tensor_copy(T["vt"][:, qsl], pv[0:64, :])

            def emit_vnat(T, qc):
                vp = fps.tile([128, 4, H], F32, name="vp", tag="f")
                for t in range(4):
                    st = qc * 4 + t
                    nc.tensor.matmul(
                        vp[:, t, :],
                        lhsT=T["vt"][:, st * 128:(st + 1) * 128],
                        rhs=ident[0:64, 0:64],
                        start=True, stop=True,
                    )
                nc.scalar.copy(T["vnat"][:, qc * 4:(qc + 1) * 4, 0:H], vp)

            # ---------------- attention helpers ----------------
            def scores_mm(T, kt, dst, c0, c1):
                col = slice(kt * 128, (kt + 1) * 128)
                if kt % 2 == 0:
                    nc.tensor.matmul(
                        dst, lhsT=T["qk_b"][0:64, col],
                        rhs=T["qk_a"][0:64, c0:c1],
                        start=True, stop=True,
                    )
                else:
                    nc.tensor.matmul(
                        dst, lhsT=T["qk_a"][64:128, col],
                        rhs=T["qk_b"][64:128, c0:c1],
                        start=True, stop=True,
                    )

            def finalize_q(T, q, opsum):
                # drain quarter q (columns [q*512, (q+1)*512)) of O^T
                q_lo = q * QW
                nc.vector.tensor_copy(T["osb"][0:H + 1, q_lo:q_lo + QW], opsum)
                nc.sync.dma_start(
                    out=T["onat"][:, q * 4:(q + 1) * 4, :],
                    in_=T["osb"][0:80, q_lo:q_lo + QW],
                    transpose=True,
                )
                for t in range(q * 4, (q + 1) * 4):
                    nc.vector.reciprocal(T["rcp"][:, t:t + 1],
                                         T["onat"][:, t, H:H + 1])
                    nc.vector.tensor_scalar_mul(
                        T["o_out"][:, t, :], T["onat"][:, t, 0:H],
                        T["rcp"][:, t:t + 1]
                    )
                nc.sync.dma_start(
                    out=out_r[:, q * 4:(q + 1) * 4, :],
                    in_=T["o_out"][:, q * 4:(q + 1) * 4, :],
                )

            def attn_half(T, h, fillers=(), per_kt=1, interleave=None,
                          drain_early=False):
                h_lo, h_hi = h * HW_, (h + 1) * HW_
                n_kt = h_hi // 128
                fillers = list(fillers)
                ops = {}
                for q in (2 * h, 2 * h + 1):
                    ops[q] = aps.tile([H + 1, QW], F32, name="opsum",
                                      tag="o", bufs=2)

                def emit_attnV(kt, at):
                    for q in (2 * h, 2 * h + 1):
                        if kt // 4 > q:
                            continue
                        nc.tensor.matmul(
                            ops[q],
                            lhsT=T["vnat"][:, kt, :],
                            rhs=at[:, q * QW:(q + 1) * QW],
                            start=(kt == 0),
                            stop=(kt == 4 * q + 3),
                            skip_group_check=True,
                        )
                    if drain_early and kt == 8 * h + 3:
                        finalize_q(T, 2 * h, ops[2 * h])

                pending = []
                for kt in range(n_kt):
                    lo = max(h_lo, kt * 128)
                    at = atn.tile([128, S], BF16, name="attn", tag="at")
                    c = lo
                    while c < h_hi:
                        ce = min(c + sp_w, h_hi)
                        if sp_share:
                            sp = fps.tile([128, sp_w], F32, name="sp",
                                          tag="f")
                        else:
                            sp = aps.tile([128, sp_w], F32, name="sp",
                                          tag="s", bufs=sp_bufs)
                        scores_mm(T, kt, sp[:, 0:ce - c], c, ce)
                        nc.scalar.activation(
                            out=at[:, c:ce], in_=sp[:, 0:ce - c],
                            func=mybir.ActivationFunctionType.Exp,
                            scale=inv_sqrt_h,
                        )
                        c = ce
                    if kt * 128 >= h_lo:  # diagonal block in this half
                        nc.vector.tensor_mul(
                            at[:, kt * 128:(kt + 1) * 128],
                            at[:, kt * 128:(kt + 1) * 128],
                            trimask,
                        )
                        if kt % 4:  # zero [dq*512, kt*128) below-diag cols
                            nc.vector.memset(
                                at[:, (kt // 4) * QW:kt * 128], 0.0
                            )
                    pending.append((kt, at))
                    if len(pending) > av_lag:
                        emit_attnV(*pending.pop(0))
                    for _ in range(per_kt):
                        if fillers:
                            fillers.pop(0)()
                    if interleave is not None and kt == 2:
                        interleave()
                for p in pending:
                    emit_attnV(*p)
                for u in fillers:
                    u()
                if drain_early:
                    finalize_q(T, 2 * h + 1, ops[2 * h + 1])
                return ops

            # ---------------- body sequencing (skewed) ----------------
            def front1_units(T):
                return ([(lambda si=si: emit_xpose(T, si))
                         for si in range(8)]
                        + [lambda: emit_proj(T, 0), lambda: emit_proj(T, 1),
                           lambda: emit_vnat(T, 0)])

            def front2_units(T):
                return ([(lambda si=si: emit_xpose(T, si))
                         for si in range(8, 16)]
                        + [lambda: emit_proj(T, 2), lambda: emit_proj(T, 3),
                           lambda: emit_vnat(T, 2)])

            def emit_bodies(n, looping=False):
                T = alloc_body_tiles()
                if phase == "load":
                    for t in range(ST):
                        nc.vector.tensor_copy(
                            T["o_out"][:, t, :],
                            T["x_bf"].rearrange("p a b -> p (a b)")
                                [:, t * H:(t + 1) * H],
                        )
                    nc.sync.dma_start(out=out_r, in_=T["o_out"])
                    return
                for u in front1_units(T):
                    u()
                for b in range(n):
                    attn_half(
                        T, 0, fillers=front2_units(T), per_kt=h0_per_kt,
                        interleave=lambda TT=T: emit_vnat(TT, 1),
                        drain_early=True,
                    )
                    if b < n - 1 or looping:
                        Tn = alloc_body_tiles()
                        attn_half(T, 1,
                                  fillers=[lambda TT=T: emit_vnat(TT, 3)]
                                          + front1_units(Tn),
                                  drain_early=True)
                        T = Tn
                    else:
                        attn_half(T, 1,
                                  fillers=[lambda TT=T: emit_vnat(TT, 3)],
                                  drain_early=True)

            def emit_loop_bodies(n):
                # cross-boundary skew: every body fully skewed; the last
                # body preps the next iteration's first body before the
                # For_i back-edge.  Requires per-iteration pool-slot counts
                # divisible by their bufs (holds: 10 bodies/iteration).
                for b in range(n):
                    T = LC["T"]
                    attn_half(
                        T, 0, fillers=front2_units(T), per_kt=h0_per_kt,
                        interleave=lambda TT=T: emit_vnat(TT, 1),
                        drain_early=True,
                    )
                    Tn = alloc_body_tiles()
                    attn_half(T, 1,
                              fillers=[lambda TT=T: emit_vnat(TT, 3)]
                                      + front1_units(Tn),
                              drain_early=True)
                    LC["T"] = Tn

            if repeat > 1:
                LC = {"T": alloc_body_tiles()}
                for u in front1_units(LC["T"]):
                    u()
                with tc.For_i(0, repeat // unroll, 1):
                    emit_loop_bodies(unroll)
            else:
                emit_bodies(nbody)
    nc.compile()
    return nc


_NC_CACHE = {}


def _get_nc(S=SEQ, D=D_EMBED):
    key = (S, D)
    if key not in _NC_CACHE:
        _NC_CACHE[key] = build_attention_nc(S, D)
    return _NC_CACHE[key]


def kernel(x, Wq, Wk, Wv):
    """Full-input entry point: x [8, 2048, 1024] f32 -> [8, 2048, 64] f32."""
    from concourse.bass_utils import run_bass_kernel_spmd

    x = np.asarray(x, dtype=np.float32)
    Wq = np.ascontiguousarray(np.asarray(Wq, dtype=np.float32))
    Wk = np.ascontiguousarray(np.asarray(Wk, dtype=np.float32))
    Wv = np.ascontiguousarray(np.asarray(Wv, dtype=np.float32))
    assert x.shape == (BATCH, SEQ, D_EMBED), x.shape

    nc = _get_nc()
    in_maps = [
        {"x": np.ascontiguousarray(x[b]), "Wq": Wq, "Wk": Wk, "Wv": Wv}
        for b in range(BATCH)
    ]
    res = run_bass_kernel_spmd(nc, in_maps, core_ids=list(range(N_CORES)))
    return np.stack([res.results[b]["out"] for b in range(BATCH)], axis=0)
